# revision 6
# baseline (speedup 1.0000x reference)
"""Trainium2 Bass kernel for nn_DiscreteStateSpaceModel_77077483094247.

Math: the reference computes y = einsum('nij,ijk->nik', u, K) but only uses
y[:, -1, :], so the whole model collapses to

    out = (u_t[:,-1,:] @ W_in.T + b_in) @ (C @ A_d^1023 @ B_d) @ W_out.T + b_out

A_d = expm(-0.01*HiPPO) is lower triangular, so G = A_d^1023 is lower
triangular and its (validated) dominant block is G00 = (A_d[:128,:128])^1023;
the coupling/lower blocks are ~1e-6 of it.  Hence with
A := A_d[:128,:128], C1 := C[:,:128], Btop := B_d[:128,:]:

    wb  = u_last @ W_in^T @ C1 + b_in^T C1          [2, 128]
    v   = wb @ A^1023                               [2, 128]
    out = v @ (Btop @ W_out^T) + b_out              [2, 512]

A^1023 = prod_k A^(2^k), k=0..9.  The kernel runs the 9-step squaring
chain keeping (S_k, S_k^T) pairs (each step: two 128x128 matmuls + one
PSUM->SBUF copy), and folds each power into the 2-row vector as soon as
it is produced: rT_{k+1} = MM(lhsT=S_k, rhs=rT_k) - a 2-column matmul
that rides in the chain's sync gaps.  All weight transposes are done
host-side (layout prep only), so the PE never runs a transpose.

Sharding: u_t sharded over batch (2 rows/core); small matrices replicated;
the chain duplicated per core (per the spec hint).

matmul computes lhsT.T @ rhs; fp32 everywhere (float32r's 11-bit mantissa
amplifies ~400x through the squaring chain - measured 9e-2 rel err).
"""

import numpy as np
from contextlib import ExitStack

from concourse import bacc, bass, mybir, tile
from concourse import bass_utils

B_SZ, SEQ, D_IN, H_DIM, D_OUT = 16, 1024, 512, 256, 512
N_CORES = 8
B_LOC = B_SZ // N_CORES  # 2 batch rows per core

F32 = mybir.dt.float32
P = 128  # partitions / chain block size
H2 = D_OUT // 2


def _build():
    nc = bacc.Bacc("TRN2", target_bir_lowering=False, debug=False,
                   num_devices=N_CORES)

    a00 = nc.dram_tensor("a00", [P, P], F32, kind="ExternalInput")
    a00t = nc.dram_tensor("a00t", [P, P], F32, kind="ExternalInput")
    wint = nc.dram_tensor("wint", [D_IN, H_DIM], F32, kind="ExternalInput")
    c1 = nc.dram_tensor("c1", [H_DIM, P], F32, kind="ExternalInput")
    btt = nc.dram_tensor("btt", [H_DIM, P], F32, kind="ExternalInput")
    wot = nc.dram_tensor("wot", [H_DIM, D_OUT], F32, kind="ExternalInput")
    ult = nc.dram_tensor("ult", [D_IN, B_LOC], F32, kind="ExternalInput")
    bin_ = nc.dram_tensor("bin", [1, H_DIM], F32, kind="ExternalInput")
    bout = nc.dram_tensor("bout", [1, D_OUT], F32, kind="ExternalInput")
    ones2 = nc.dram_tensor("ones2", [1, B_LOC], F32, kind="ExternalInput")
    out = nc.dram_tensor("out", [B_LOC, D_OUT], F32, kind="ExternalOutput")

    with tile.TileContext(nc) as tc, ExitStack() as ctx:
        const = ctx.enter_context(tc.tile_pool(name="const", bufs=1))
        work = ctx.enter_context(tc.tile_pool(name="work", bufs=1))
        psum = ctx.enter_context(
            tc.tile_pool(name="psum", bufs=2, space=bass.MemorySpace.PSUM))

        V = nc.vector
        S = nc.scalar
        MM = nc.tensor.matmul

        # ---- DMA loads: a00/a00t first on their engines ------------------
        xa = [work.tile([P, 2 * P], F32, tag=f"xa{k}", name=f"xa{k}")
              for k in range(10)]
        nc.sync.dma_start(xa[0][:, 0:P], a00.ap()[:, :])
        nc.scalar.dma_start(xa[0][:, P:2 * P], a00t.ap()[:, :])

        wint_sb = const.tile([P, 4, H_DIM], F32, tag="wint")
        nc.sync.dma_start(wint_sb[:],
                          wint.ap().rearrange("(ko p) h -> p ko h", p=P))
        c1_sb = const.tile([P, 2, P], F32, tag="c1")
        nc.scalar.dma_start(c1_sb[:],
                            c1.ap().rearrange("(hb p) f -> p hb f", p=P))
        ult_sb = const.tile([P, 4, B_LOC], F32, tag="ult")
        nc.gpsimd.dma_start(ult_sb[:],
                            ult.ap().rearrange("(ko p) n -> p ko n", p=P))
        bin_sb = const.tile([1, H_DIM], F32, tag="bin")
        nc.gpsimd.dma_start(bin_sb[:], bin_.ap()[:, :])
        ones2_sb = const.tile([1, B_LOC], F32, tag="ones2")
        nc.gpsimd.dma_start(ones2_sb[:], ones2.ap()[:, :])
        bout_sb = const.tile([1, D_OUT], F32, tag="bout")
        nc.gpsimd.dma_start(bout_sb[:], bout.ap()[:, :])
        btt_sb = const.tile([P, 2, P], F32, tag="btt")
        nc.sync.dma_start(btt_sb[:],
                          btt.ap().rearrange("(hb p) f -> p hb f", p=P))
        wot_sb = const.tile([P, 2, D_OUT], F32, tag="wot")
        nc.scalar.dma_start(wot_sb[:],
                            wot.ap().rearrange("(hb p) d -> p hb d", p=P))

        warm_sb = work.tile([1, B_LOC], F32, tag="warm")
        S.copy(warm_sb[:], ones2_sb[:])

        # ---- chain-independent small jobs (filler between chain iters) ---
        g_sb = work.tile([P, 2, B_LOC], F32, tag="g")
        wt_sb = work.tile([P, B_LOC], F32, tag="wt")
        rt_sb = [work.tile([P, B_LOC], F32, tag=f"rt{k}", name=f"rt{k}")
                 for k in range(11)]
        d_sb = work.tile([P, D_OUT], F32, tag="d")
        g_ps = psum.tile([P, 2, B_LOC], F32, tag="small", bufs=1)
        w_ps = psum.tile([P, B_LOC], F32, tag="small", bufs=1)
        d_ps = psum.tile([P, D_OUT], F32, tag="d", bufs=1)
        out_ps = psum.tile([B_LOC, 2, H2], F32, tag="o", bufs=1)
        out_ps0 = out_ps[:, 0, :]
        out_ps1 = out_ps[:, 1, :]

        def g_job(hb, part):
            # g[:,hb,:] = b_in[hb-block]^T x ones + W_in-chunks @ u_last^T
            def go():
                if part == 0:
                    MM(g_ps[:, hb, :], bin_sb[0:1, P * hb:P * (hb + 1)],
                       ones2_sb[:], start=True, stop=False)
                    for ko in range(2):
                        MM(g_ps[:, hb, :], wint_sb[:, ko, P * hb:P * (hb + 1)],
                           ult_sb[:, ko, :], start=False, stop=False)
                else:
                    for ko in range(2, 4):
                        MM(g_ps[:, hb, :], wint_sb[:, ko, P * hb:P * (hb + 1)],
                           ult_sb[:, ko, :], start=False, stop=(ko == 3))
                    if hb == 1:
                        S.copy(g_sb[:], g_ps[:])
            return go

        def wt_job():
            # wT = C1^T @ g  ( = wb^T, the r-chain start rT_0 )
            for hb in range(2):
                MM(w_ps[:], c1_sb[:, hb, :], g_sb[:, hb, :],
                   start=(hb == 0), stop=(hb == 1))
            S.copy(rt_sb[0][:], w_ps[:])

        def seed_job():
            MM(out_ps0[:], ones2_sb[:], bout_sb[:, 0:H2],
               start=True, stop=False)
            MM(out_ps1[:], ones2_sb[:], bout_sb[:, H2:D_OUT],
               start=True, stop=False)

        r_ps = [psum.tile([P, B_LOC], F32, tag="r", bufs=1,
                          name=f"rps{k}")
                for k in range(10)]

        def r_job(k):
            # rT_{k+1} = S_k^T... MM(lhsT=S_k, rhs=rT_k) = (r_k @ S_k)^T
            def go():
                MM(r_ps[k][:], xa[k][:, 0:P], rt_sb[k][:],
                   start=True, stop=True)
                S.copy(rt_sb[k + 1][:], r_ps[k][:])
            return go

        def d_job(ko):
            # D = Btop @ W_out^T
            def go():
                MM(d_ps[:], btt_sb[:, ko, :], wot_sb[:, ko, :],
                   start=(ko == 0), stop=(ko == 1))
                if ko == 1:
                    V.tensor_copy(d_sb[:], d_ps[:])
            return go

        jobs = [g_job(0, 0), g_job(0, 1), g_job(1, 0), g_job(1, 1),
                wt_job, seed_job,
                r_job(0), r_job(1), r_job(2), r_job(3), r_job(4),
                r_job(5), r_job(6), d_job(0), r_job(7), d_job(1), r_job(8)]
        # per-gap job counts after chain iters 1..9
        gap_plan = [0, 0, 2, 2, 2, 2, 3, 2, 4]

        def emit_jobs(n):
            for _ in range(n):
                if jobs:
                    jobs.pop(0)()

        # ---- squaring chain: S_{k+1}=S_k@S_k, T_{k+1}=S_{k+1}^T ----------
        for k in range(9):
            ps = psum.tile([P, 2 * P], F32, tag="chain", bufs=2)
            MM(ps[:, 0:P], xa[k][:, P:2 * P], xa[k][:, 0:P],
               start=True, stop=True)
            if k < 8:
                MM(ps[:, P:2 * P], xa[k][:, 0:P], xa[k][:, P:2 * P],
                   start=True, stop=True)
                V.tensor_copy(xa[k + 1][:], ps[:])
            else:
                V.tensor_copy(xa[9][:, 0:P], ps[:, 0:P])
            emit_jobs(gap_plan[k])
        emit_jobs(len(jobs))

        # ---- tail: r10 = vT, out = v @ D + bias-seed ---------------------
        MM(r_ps[9][:], xa[9][:, 0:P], rt_sb[9][:], start=True, stop=True)
        S.copy(rt_sb[10][:], r_ps[9][:])

        out_sb = work.tile([B_LOC, D_OUT], F32, tag="osb")
        MM(out_ps0[:], rt_sb[10][:], d_sb[:, 0:H2], start=False, stop=True)
        V.tensor_copy(out_sb[:, 0:H2], out_ps0[:])
        nc.sync.dma_start(out.ap()[:, 0:H2], out_sb[:, 0:H2])
        MM(out_ps1[:], rt_sb[10][:], d_sb[:, H2:D_OUT], start=False, stop=True)
        V.tensor_copy(out_sb[:, H2:D_OUT], out_ps1[:])
        nc.scalar.dma_start(out.ap()[:, H2:D_OUT], out_sb[:, H2:D_OUT])

    nc.compile()
    return nc


_NC_CACHE = {}


def _get_nc():
    if "nc" not in _NC_CACHE:
        _NC_CACHE["nc"] = _build()
    return _NC_CACHE["nc"]


_ONES2 = np.ones((1, B_LOC), dtype=np.float32)


def kernel(u_t, W_in, b_in, C, W_out, b_out, A_d, B_d, **run_kwargs):
    nc = _get_nc()
    u_t = np.asarray(u_t, dtype=np.float32)
    A_d = np.asarray(A_d, dtype=np.float32)
    shared = {
        "a00": np.ascontiguousarray(A_d[0:P, 0:P]),
        "a00t": np.ascontiguousarray(A_d[0:P, 0:P].T),
        "wint": np.ascontiguousarray(np.asarray(W_in, dtype=np.float32).T),
        "c1": np.ascontiguousarray(np.asarray(C, dtype=np.float32)[:, 0:P]),
        "btt": np.ascontiguousarray(np.asarray(B_d, dtype=np.float32)[0:P, :].T),
        "wot": np.ascontiguousarray(np.asarray(W_out, dtype=np.float32).T),
        "bin": np.ascontiguousarray(
            np.asarray(b_in, dtype=np.float32)[None, :]),
        "bout": np.ascontiguousarray(
            np.asarray(b_out, dtype=np.float32)[None, :]),
        "ones2": _ONES2,
    }
    in_maps = []
    for i in range(N_CORES):
        m = dict(shared)
        m["ult"] = np.ascontiguousarray(
            u_t[i * B_LOC:(i + 1) * B_LOC, SEQ - 1, :].T)
        in_maps.append(m)
    res = bass_utils.run_bass_kernel_spmd(
        nc, in_maps, core_ids=list(range(N_CORES)), **run_kwargs)
    out = np.concatenate([res.results[i]["out"] for i in range(N_CORES)], axis=0)
    if run_kwargs:
        return out, res
    return out
